# revision 14
# baseline (speedup 1.0000x reference)
"""Causal self-attention, head-tensor-parallel across 8 TRN2 NeuronCores.

Problem: x[2,2048,1024] -> qkv = x@W_attn+b_attn -> 16-head causal attention
(head dim 64) -> y@W_proj+b_proj.

Sharding: heads are tensor-parallel. Core c owns heads 2c and 2c+1:
  - W_attn column slices for its q/k/v features (384 cols), W_proj row slice
    (128 rows). Every core reads all of x (transposed+bf16 on host).
  - Each core emits a full [4096,1024] fp16 partial of the output projection;
    the host sums the 8 partials and adds b_proj.

On-core dataflow (all matmuls bf16 in / fp32 PSUM accum):
  1. qkv^T[384,4096] = W_slice^T @ x^T (features on partitions).  q,k are
     computed for all of a batch's chunks FIRST so the S matmuls (and the
     ScalarE exp stream) start ~14us in; v / batch-1 / V-transposes fill PE
     gaps underneath the exp-paced attention phase.
  2. V = PE-transpose of v^T, augmented with a ones column (row sums of
     P fall out of the AV matmul as column 64 -> softmax denominator).
  3. S^T[k,q] = k^T.T @ q^T per head, causally block-skipped; the two heads
     run row-group-packed (contraction K=64 at partitions 0-63 / 64-127).
     exp via ScalarE with scale=1/8, bf16 out = P^T (ragged per-kt strips
     packed contiguously; exp runs in uniform [128,1024] windows).  Head-
     alternating rotation through a 2-buf PSUM pool keeps exp at full rate
     in 4 banks.  Diagonal blocks get a triu mask multiply after exp.
  4. y_aug^T[65,512] per (head, 512-q chunk): V_aug stationary, P^T moving,
     accumulated over kt in PSUM.  Normalized IN TRANSPOSED ORIENTATION
     (no PE transposes): denom row 64 -> partition 0 (DVE copy),
     reciprocal_approx_fast (DVE), partition_broadcast to 64 rows (GpSimd),
     DVE multiply -> yT_sb[h*64:(h+1)*64, tok] bf16.
  5. out[tok,1024] = yT.T @ W_proj per 128-token tile (interleaved per qc
     right behind the normalize), evict fp16, DMA.
"""

import numpy as np
import ml_dtypes

import concourse.bacc as bacc
import concourse.bass as bass
import concourse.mybir as mybir
import concourse.tile as tile
from concourse.bass_utils import run_bass_kernel_spmd
from concourse.masks import make_identity

BF16 = mybir.dt.bfloat16
FP16 = mybir.dt.float16
FP32 = mybir.dt.float32

B, T, C, H = 2, 2048, 1024, 16
D = C // H            # 64
N_CORES = 8
HPC = H // N_CORES    # heads per core = 2
TOK = B * T           # 4096
P = 128               # partitions / tile edge
KT = T // P           # 16 k/q tiles per batch element
NQ = 1024             # S^T / exp chunk width (2 PSUM banks)
XC = 512              # x^T token chunk for streaming
NCHB = T // XC        # chunks per batch = 4
CW = (C // P) * XC    # flat per-partition chunk width = 4096

bf16 = ml_dtypes.bfloat16


def _pt_offsets():
    """Column offsets of each k-tile's ragged [k, q] strip in the P^T store."""
    offs, total = [], 0
    for t in range(KT):
        offs.append(total)
        total += T - P * t
    return offs, total


PT_OFF, PT_COLS = _pt_offsets()  # PT_COLS = 17408


def build_nc():
    # Bacc (not raw Bass): its lowering legalizes TRN2's one-wait-per-
    # instruction constraint by splitting multi-waits into EventSemaphores.
    nc = bacc.Bacc("TRN2", target_bir_lowering=False, debug=False)

    # All DRAM inputs host-prepacked so every DMA is a flat 2D pattern with
    # multi-KB contiguous runs per partition (1KB packets from 3D patterns
    # were the startup bottleneck).
    xT = nc.dram_tensor("xT", [TOK // XC, P, CW], BF16, kind="ExternalInput").ap()
    w_qkv = nc.dram_tensor(
        "w_qkv", [P, (C // P) * 3 * P], BF16, kind="ExternalInput"
    ).ap()
    b_qkv = nc.dram_tensor("b_qkv", [P, 3], FP32, kind="ExternalInput").ap()
    w_p = nc.dram_tensor("w_p", [P, C], BF16, kind="ExternalInput").ap()
    out_p = nc.dram_tensor("out_p", [TOK, C], FP16, kind="ExternalOutput").ap()

    with tile.TileContext(nc) as tc:
        _emit(nc, tc, xT, w_qkv, b_qkv, w_p, out_p)
    nc.compile()
    return nc


def _emit(nc, tc, xT, w_qkv, b_qkv, w_p, out_p):
    from contextlib import ExitStack

    ctx = ExitStack()
    with ctx:
        consts = ctx.enter_context(tc.tile_pool(name="consts", bufs=1))
        persist = ctx.enter_context(tc.tile_pool(name="persist", bufs=1))

        # ---- constants ----
        w_qkv_sb = consts.tile([P, (C // P) * 3 * P], BF16)
        nc.sync.dma_start(out=w_qkv_sb, in_=w_qkv)
        bias_sb = consts.tile([P, 3], FP32)  # col m: bias of feature m*128+p
        nc.sync.dma_start(out=bias_sb, in_=b_qkv)
        w_p_sb = consts.tile([P, C], BF16)
        nc.sync.dma_start(out=w_p_sb, in_=w_p)
        ident = consts.tile([P, P], BF16)
        make_identity(nc, ident)

        # ---- persistent activations ----
        qT = persist.tile([P, TOK], BF16)   # rows: head A dims 0-63, head B 64-127
        kTt = persist.tile([P, TOK], BF16)
        vT = persist.tile([P, TOK], BF16)
        qkvT = [qT, kTt, vT]
        # V augmented with ones column, per (b, head): [k-in-tile, ktile, D+1]
        v_aug = [
            [persist.tile([P, KT, D + 1], BF16, name=f"v_aug_{b}_{h}") for h in range(HPC)]
            for b in range(B)
        ]
        yT_sb = persist.tile([P, TOK], BF16)    # normalized y^T, feat on partitions
        # ragged P^T store, one per head (reused across b; Tile's slice-level
        # deps let exp(b1) windows start once AV(b0) readers clear)
        pt_sb = [persist.tile([P, PT_COLS], BF16, name=f"pt_{h}") for h in range(HPC)]

        # ---- pools ----
        osb = ctx.enter_context(tc.tile_pool(name="o_sb", bufs=4))
        rcp = ctx.enter_context(tc.tile_pool(name="rc_rows", bufs=4))
        bcp = ctx.enter_context(tc.tile_pool(name="bcast", bufs=4))
        # deferred-v means a batch's 4 x-chunks stay live until its v pass
        xp = tc.alloc_tile_pool(name="xT_pool", bufs=5)
        # PSUM phase A: s(4) + qkv(2) + vt(2) = 8 banks; s sits at the bottom
        # of the stack so releasing qkv+vt frees the top for ya+o in phase B:
        # s(4) + ya(2) + o(2) = 8 banks.
        sps = tc.alloc_tile_pool(name="s_ps", bufs=2, space="PSUM")
        qps = tc.alloc_tile_pool(name="qkv_ps", bufs=2, space="PSUM")
        vtp = tc.alloc_tile_pool(name="vt_ps", bufs=2, space="PSUM")

        x_chunks = {}

        def emit_qkv(nch, mis):
            if nch not in x_chunks:
                x_sb = xp.tile([P, CW], BF16, name="x_sb")
                nc.sync.dma_start(out=x_sb, in_=xT[nch])
                x_chunks[nch] = x_sb
            x_sb = x_chunks[nch]
            for mi in mis:
                ps = qps.tile([P, XC], FP32, name="qkv_acc")
                for kt in range(C // P):
                    nc.tensor.matmul(
                        ps,
                        w_qkv_sb[:, kt * 3 * P + mi * P : kt * 3 * P + (mi + 1) * P],
                        x_sb[:, kt * XC : (kt + 1) * XC],
                        start=(kt == 0),
                        stop=(kt == C // P - 1),
                    )
                nc.vector.tensor_scalar_add(
                    out=qkvT[mi][:, nch * XC : (nch + 1) * XC],
                    in0=ps,
                    scalar1=bias_sb[:, mi : mi + 1],
                )

        def emit_v(b):
            for h in range(HPC):
                nc.vector.memset(v_aug[b][h][:, :, D : D + 1], 1.0)
            for kt in range(KT):
                tok0 = b * T + kt * P
                ps_t = vtp.tile([P, P], BF16, name="vt_t")
                nc.tensor.transpose(ps_t, vT[:, tok0 : tok0 + P], ident)
                for h in range(HPC):
                    nc.vector.tensor_copy(
                        out=v_aug[b][h][:, kt, 0:D],
                        in_=ps_t[:, h * D : (h + 1) * D],
                    )

        def emit_s(b):
            # S^T / exp over the PACKED column space of the P^T store: the
            # causal strips are contiguous, so exp runs in uniform
            # [128, 1024] windows (17408 = 17*1024). Heads alternate through
            # the 2-buf PSUM pool: fill(A,w) | exp(A,w) overlaps fill(B,w).
            emitted_mask = [set() for _ in range(HPC)]
            for w in range(PT_COLS // NQ):
                w0, w1 = w * NQ, (w + 1) * NQ
                for h in range(HPC):
                    ps_s = sps.tile([P, NQ], FP32, name="s_acc")
                    rows = slice(h * D, (h + 1) * D)
                    for kt in range(KT):
                        a = max(w0, PT_OFF[kt])
                        bnd = min(w1, PT_OFF[kt] + (T - P * kt))
                        if a >= bnd:
                            continue
                        ktok = b * T + kt * P
                        # split at PSUM bank (512) boundaries within the window
                        c = a
                        while c < bnd:
                            nxt = min(bnd, w0 + ((c - w0) // 512 + 1) * 512)
                            q0 = kt * P + (c - PT_OFF[kt])
                            nc.tensor.matmul(
                                ps_s[:, c - w0 : nxt - w0],
                                kTt[rows, ktok : ktok + P],
                                qT[rows, b * T + q0 : b * T + q0 + nxt - c],
                                start=True,
                                stop=True,
                                tile_position=(h * D, 0),
                            )
                            c = nxt
                    nc.scalar.activation(
                        out=pt_sb[h][:, w0:w1],
                        in_=ps_s,
                        func=mybir.ActivationFunctionType.Exp,
                        scale=1.0 / np.sqrt(D),
                    )
                    # causal masks for diagonal blocks fully covered so far
                    for kt in range(KT):
                        if kt in emitted_mask[h] or PT_OFF[kt] + P > w1:
                            continue
                        emitted_mask[h].add(kt)
                        nc.gpsimd.affine_select(
                            out=pt_sb[h][:, PT_OFF[kt] : PT_OFF[kt] + P],
                            in_=pt_sb[h][:, PT_OFF[kt] : PT_OFF[kt] + P],
                            pattern=[[1, P]],
                            compare_op=mybir.AluOpType.is_ge,
                            fill=0.0,
                            base=0,
                            channel_multiplier=-1,
                        )

        QQ = 512  # AV accumulator width (1 PSUM bank per head-chunk)

        def emit_proj(gq, evict):
            """Projection chunk for 128-token tile gq from yT_sb, evict+store."""
            o_sb = osb.tile([P, C], FP16, name="o_stage")
            for fj in range(C // 512):
                ps_o = ops.tile([P, 512], FP32, name="o_acc")
                nc.tensor.matmul(
                    ps_o,
                    yT_sb[:, gq * P : (gq + 1) * P],
                    w_p_sb[:, fj * 512 : (fj + 1) * 512],
                    start=True,
                    stop=True,
                )
                evict(out=o_sb[:, fj * 512 : (fj + 1) * 512], in_=ps_o)
            nc.sync.dma_start(out=out_p[gq * P : (gq + 1) * P, :], in_=o_sb)

        def emit_av(b, evict):
            # AV in y^T orientation: V_aug stationary, P^T moving ->
            # y^T_aug[65, 512] accumulated over kt in PSUM, per 512-q chunk.
            # Normalize without transposing: recip row 64, broadcast, multiply.
            # Projection tiles follow per qc right behind the normalize.
            for qc in range(T // QQ):
                q0, q1 = qc * QQ, (qc + 1) * QQ
                kmax = q1 // P - 1
                for h in range(HPC):
                    ps_ya = yap.tile([D + 1, QQ], FP32, name="yta")
                    for kt in range(kmax + 1):
                        sub0 = max(q0, kt * P)
                        col0 = PT_OFF[kt] + sub0 - kt * P
                        nc.tensor.matmul(
                            ps_ya[:, sub0 - q0 : QQ],
                            v_aug[b][h][:, kt, :],
                            pt_sb[h][:, col0 : col0 + q1 - sub0],
                            start=(kt == 0),
                            stop=(kt == kmax),
                        )
                    # denom row to partition 0 (the fused-ucode reciprocal
                    # requires base-partition-0 operands), then fast recip
                    dn = rcp.tile([1, QQ], FP32, name="dn_row")
                    nc.vector.tensor_copy(out=dn, in_=ps_ya[D : D + 1, :])
                    rc = rcp.tile([1, QQ], FP32, name="rc_row")
                    nc.vector.reciprocal_approx_fast(rc, dn)
                    bc = bcp.tile([D, QQ], FP32, name="bc")
                    nc.gpsimd.partition_broadcast(bc, rc, channels=D)
                    nc.vector.tensor_tensor(
                        out=yT_sb[h * D : (h + 1) * D, b * T + q0 : b * T + q1],
                        in0=ps_ya[0:D, :],
                        in1=bc,
                        op=mybir.AluOpType.mult,
                    )
                for j in range(QQ // P):
                    emit_proj(b * KT + qc * (QQ // P) + j, evict)

        # ---- pipeline (emission order = scheduling priority) ----
        for nch in range(NCHB):
            emit_qkv(nch, (0, 1))       # q,k of batch 0 first: exp starts ~14us
        emit_s(0)
        for nch in range(NCHB):
            emit_qkv(nch, (2,))         # v(b0) + everything below: PE gap fill
        emit_v(0)
        for nch in range(NCHB, 2 * NCHB):
            emit_qkv(nch, (0, 1, 2))
        emit_v(1)
        vtp.release()
        qps.release()
        xp.release()
        yap = tc.alloc_tile_pool(name="ya_ps", bufs=2, space="PSUM")
        ops = tc.alloc_tile_pool(name="o_ps", bufs=2, space="PSUM")
        emit_av(0, nc.vector.tensor_copy)
        emit_s(1)

        def evict_b1(out, in_):
            evict_b1.i += 1
            if evict_b1.i % 2 == 0:
                nc.vector.tensor_copy(out=out, in_=in_)
            else:
                nc.scalar.copy(out=out, in_=in_)

        evict_b1.i = 0
        emit_av(1, evict_b1)
        ops.release()
        yap.release()
        sps.release()


def shard_inputs(x, W_attn, b_attn, W_proj, b_proj):
    x = np.asarray(x, np.float32)
    W_attn = np.asarray(W_attn, np.float32)
    b_attn = np.asarray(b_attn, np.float32)
    W_proj = np.asarray(W_proj, np.float32)

    # [chunk, p, ktile*tok]: flat contiguous per-partition DMA source for x^T
    xT = np.ascontiguousarray(
        x.reshape(TOK // XC, XC, C // P, P).transpose(0, 3, 2, 1).reshape(
            TOK // XC, P, CW
        )
    ).astype(bf16)
    in_maps = []
    for c in range(N_CORES):
        w_cols = np.stack(
            [W_attn[:, m * C + P * c : m * C + P * (c + 1)] for m in range(3)],
            axis=1,
        )  # [C, 3, P]
        # [p, kt*3*P]: row kt*128+p of the (q|k|v) 384-col slice
        w_slice = np.ascontiguousarray(
            w_cols.reshape(C // P, P, 3 * P).transpose(1, 0, 2).reshape(P, -1)
        ).astype(bf16)
        b_slice = np.ascontiguousarray(
            np.stack(
                [b_attn[m * C + P * c : m * C + P * (c + 1)] for m in range(3)],
                axis=1,
            )
        ).astype(np.float32)  # [P, 3]
        wp_slice = np.ascontiguousarray(W_proj[P * c : P * (c + 1), :]).astype(bf16)
        in_maps.append(
            {"xT": xT, "w_qkv": w_slice, "b_qkv": b_slice, "w_p": wp_slice}
        )
    return in_maps


def kernel(x, W_attn, b_attn, W_proj, b_proj, _trace=False):
    in_maps = shard_inputs(x, W_attn, b_attn, W_proj, b_proj)
    nc = build_nc()
    res = run_bass_kernel_spmd(nc, in_maps, list(range(N_CORES)), trace=_trace)
    acc = np.zeros((TOK, C), np.float64)
    for r in res.results:
        acc += r["out_p"].astype(np.float64)
    out = acc.astype(np.float32) + np.asarray(b_proj, np.float32)[None, :]
    if _trace:
        kernel.last_results = res
    return out.reshape(B, T, C)
